# revision 10
# baseline (speedup 1.0000x reference)
"""LIF-with-residue Trainium2 kernel (v7).

Reference semantics (T=4, THRESH=1, TAU=1, ALPHA=0.5):
    x: [32, 1024, 512] fp32 -> flat timeline [128 steps, 256, 512]
    per step t:
        mem   = mem + x_t
        sp    = (mem >= 1.0)
        res   = 0.5 * res + sp          # output at step t
        mem   = mem * (1 - sp)

Per core: 16384 neurons = 128 partitions x 128 f, 128 steps.
Two fused custom-DVE ops per chunk (self-overlapping ring APs stream a
whole chunk's recurrence at 1 elem/cycle):
    LIF_STEP_ANT:  memb_t = memb_{t-1} * (memb_{t-1} < 1) + x_t  (fp32)
    RES_STEP_ANT:  res_t  = res_{t-1} * 0.5 + (memb_t >= 1)      (bf16)

v7 structure:
 - per-chunk x tiles in a 3-deep rotating pool; ALL input DMAs issued
   up-front on the Sync queue (no compute-dependent waits except the
   rotation), so the input stream saturates HBM (~430 GB/s burst).
 - output DMAs grouped and deferred until the input stream is done, so
   they never steal bandwidth from the latency-critical inputs.
 - small leading chunks (fast DVE start), small trailing chunks (short
   drain tail).

Sharding: neuron n_core = p*128 + f; core c owns neurons
[c*16384, (c+1)*16384) -- data-parallel, no cross-core comms.
"""

import numpy as np

N_STEPS = 128
N_NEURONS = 131072
N_CORES = 8
N_PER_CORE = N_NEURONS // N_CORES   # 16384
P = 128                             # SBUF partitions
F = N_PER_CORE // P                 # 128 neurons per partition

_CACHE = {}

IN_CHUNKS = [2, 2, 4, 8, 16, 32, 32, 16, 8, 8]
assert sum(IN_CHUNKS) == N_STEPS
STARTS = np.cumsum([0] + IN_CHUNKS).tolist()
# output DMA after RES of chunk index -> step range [lo, hi)
OUT_AFTER = {6: (0, 96), 7: (96, 112), 8: (112, 120), 9: (120, 128)}


def _register_ops():
    """Register the two fused custom DVE ops (idempotent)."""
    import concourse.dve_ops as dve_ops
    from concourse.dve_spec import C0, C1, Spec, Src0, Src1, lower
    from concourse.dve_uop import DveOpSpec

    def reg(name, spec):
        for o in dve_ops.OPS:
            if o.name == name:
                return o
        row = max(dve_ops._SUB_OPCODE_FOR_NAME.values()) + 1
        assert row < 0x20
        shas = {}
        for ver in ("v3", "v4"):
            d = DveOpSpec(name=name, opcode=row, uops=lower(spec, ver=ver),
                          rd1_en=True)
            shas[ver] = d.sha(ver)
        op = dve_ops.DveOp(name, spec, subdim=False, uops_sha=shas)
        dve_ops.OPS.append(op)
        dve_ops.CUSTOM_DVE_SPECS[name] = spec
        dve_ops._SUB_OPCODE_FOR_NAME[name] = row
        return op

    lif = reg(
        "LIF_STEP_ANT",
        Spec(
            body=Src0 * (Src0 < C0) + Src1,
            reference=lambda in0, in1, s0, s1, imm2: (
                in0 * (in0 < s0) + in1
            ).astype(np.float32),
        ),
    )
    res = reg(
        "RES_STEP_ANT",
        Spec(
            body=Src0 * C0 + (Src1 >= C1),
            reference=lambda in0, in1, s0, s1, imm2: (
                in0 * s0 + (in1 >= s1)
            ).astype(np.float32),
        ),
    )
    return lif, res


def _build_program():
    import concourse.bacc as bacc
    import concourse.mybir as mybir
    from concourse.tile import TileContext

    f32 = mybir.dt.float32
    bf16 = mybir.dt.bfloat16
    lif, res = _register_ops()

    nc = bacc.Bacc()
    x_d = nc.dram_tensor("x", [P, N_STEPS * F], f32, kind="ExternalInput")
    o_d = nc.dram_tensor("o", [P, N_STEPS * F], bf16, kind="ExternalOutput")

    wmax = max(IN_CHUNKS)

    with TileContext(nc) as tc:
        with (
            tc.tile_pool(name="xin", bufs=3) as xpool,
            tc.tile_pool(name="single", bufs=1) as spool,
        ):
            M = spool.tile([P, N_STEPS + 1, F], f32)   # membrane ring
            R = spool.tile([P, N_STEPS + 1, F], bf16)  # residue ring
            nc.vector.memset(M[:, 0, :], 0.0)
            nc.vector.memset(R[:, 0, :], 0.0)

            xts = []
            for ci, w in enumerate(IN_CHUNKS):
                t0 = STARTS[ci]
                xt = xpool.tile([P, wmax, F], f32, name="xt", tag="xt")
                nc.sync.dma_start(
                    out=xt[:, :w, :], in_=x_d[:, t0 * F:(t0 + w) * F]
                )
                xts.append(xt)

            for ci, w in enumerate(IN_CHUNKS):
                t0 = STARTS[ci]
                nc.vector._custom_dve(
                    lif, out=M[:, t0 + 1:t0 + 1 + w, :],
                    in0=M[:, t0:t0 + w, :], in1=xts[ci][:, :w, :], s0=1.0,
                )
                nc.vector._custom_dve(
                    res, out=R[:, t0 + 1:t0 + 1 + w, :],
                    in0=R[:, t0:t0 + w, :],
                    in1=M[:, t0 + 1:t0 + 1 + w, :], s0=0.5, s1=1.0,
                )
                if ci in OUT_AFTER:
                    lo, hi = OUT_AFTER[ci]
                    nc.sync.dma_start(
                        out=o_d[:, lo * F:hi * F],
                        in_=R[:, lo + 1:hi + 1, :],
                    )
    nc.finalize()
    return nc


def _get_program():
    if "nc" not in _CACHE:
        _CACHE["nc"] = _build_program()
    return _CACHE["nc"]


def _shard_inputs(x: np.ndarray) -> list[np.ndarray]:
    """[32,1024,512] -> per-core [P, N_STEPS*F] partition-major arrays."""
    xf = np.ascontiguousarray(x, dtype=np.float32).reshape(N_STEPS, N_NEURONS)
    shards = []
    for c in range(N_CORES):
        s = xf[:, c * N_PER_CORE:(c + 1) * N_PER_CORE]   # [T, 16384]
        s = s.reshape(N_STEPS, P, F).transpose(1, 0, 2).reshape(
            P, N_STEPS * F
        )
        shards.append(np.ascontiguousarray(s))
    return shards


def _unshard_outputs(outs: list[np.ndarray]) -> np.ndarray:
    """Per-core o [P, T*F] bf16 (t-major) -> [32,1024,512] f32."""
    full = np.empty((N_STEPS, N_NEURONS), dtype=np.float32)
    for c, o in enumerate(outs):
        s = np.asarray(o).astype(np.float32).reshape(P, N_STEPS, F)
        full[:, c * N_PER_CORE:(c + 1) * N_PER_CORE] = (
            s.transpose(1, 0, 2).reshape(N_STEPS, N_PER_CORE)
        )
    return full.reshape(32, 1024, 512)


def kernel(x: np.ndarray) -> np.ndarray:
    from concourse.bass_utils import run_bass_kernel_spmd

    steps, tb, d = x.shape
    assert (steps, tb, d) == (32, 1024, 512), x.shape

    in_maps = [{"x": s} for s in _shard_inputs(x)]
    nc = _get_program()
    res = run_bass_kernel_spmd(nc, in_maps, list(range(N_CORES)))
    return _unshard_outputs(
        [res.results[c]["o"] for c in range(N_CORES)]
    )


# revision 11
# speedup vs baseline: 1.1987x; 1.1987x over previous
"""LIF-with-residue Trainium2 kernel (v8).

Reference semantics (T=4, THRESH=1, TAU=1, ALPHA=0.5):
    x: [32, 1024, 512] fp32 -> flat timeline [128 steps, 256, 512]
    per step t:
        mem   = mem + x_t
        sp    = (mem >= 1.0)
        res   = 0.5 * res + sp          # output at step t
        mem   = mem * (1 - sp)

Per core: 16384 neurons = 128 partitions x 128 f, 128 steps.

With g_t = sign(mem_t - 1) (exact threshold: (g>=0.5) == (mem>=1)) and
rho_t := 2*res_t - 2, the residue recurrence is

    rho_t = 0.5*rho_{t-1} + g_t ,  rho_{-1} = -2,  res = rho/2 + 1 (host)

v8 runs it on the DVE in 2x packed mode with a PAIR-INTERLEAVED ring:
ring row k stores (rho_{2k}[f], rho_{2k+1}[f]) adjacent, so one packed
32-bit read/write handles one neuron's two consecutive steps:

    o0 = rho_{2k}   = 0.5*SRC_0_HI(rho_{2k-1}) + SRC_1(g_{2k})
    o1 = rho_{2k+1} = 0.5*o0                   + SRC_1_HI(g_{2k+1})

The intra-pair dependency chains through the pipe (PREV_ALU_OUT); the
ring write->read distance is 2F = 256 elements, above the ~100-cycle
2x-mode read-ahead hazard window measured on HW (fails <= 192 elems,
passes >= 224).  The sign tensor is produced pair-interleaved by two
strided ScalarE Sign passes (even rows / odd rows) per chunk.

Engines: DVE = LIF (fp32 1x) + RES pair op (bf16 2x); ACT = Sign x2;
Sync = all DMAs (inputs up-front, outputs per chunk).  Host output map:
res = rho*0.5 + 1 with pair de-interleave, folded into the upcast.

Sharding: neuron n_core = p*128 + f; core c owns neurons
[c*16384, (c+1)*16384) -- data-parallel, no cross-core comms.
"""

import numpy as np

N_STEPS = 128
N_NEURONS = 131072
N_CORES = 8
N_PER_CORE = N_NEURONS // N_CORES   # 16384
P = 128                             # SBUF partitions
F = N_PER_CORE // P                 # 128 neurons per partition
NPAIR = N_STEPS // 2                # 64 pair-rows

_CACHE = {}

IN_CHUNKS = [2, 2, 4, 8, 16, 32, 32, 16, 8, 8]   # all even
assert sum(IN_CHUNKS) == N_STEPS and all(w % 2 == 0 for w in IN_CHUNKS)
STARTS = np.cumsum([0] + IN_CHUNKS).tolist()
# output DMA after RES of chunk index -> step range [lo, hi), even bounds
OUT_AFTER = {4: (0, 32), 5: (32, 64), 6: (64, 96), 7: (96, 112),
             8: (112, 120), 9: (120, 128)}


def _build_res_pair_2x_uop():
    """2x uop: packed pair = one neuron's two consecutive steps.
    o0 = SRC_0_HI*C0 + SRC_1 ; o1 = o0*C0 + SRC_1_HI.
    Lanes: L0=SRC_1, L1=SRC_0_HI, L2=SRC_1_HI, L3=CONST_0, L4=o0."""
    from concourse.dve_uop import (
        AluInp as A,
        AluOp,
        DelayInp,
        InpSel,
        OutPath,
        OutSel,
        Trigger,
        UopConfig,
    )

    u = UopConfig()
    u.enable_input(InpSel.SRC_0, 0)      # rho_{2k-2}: delivered, unused
    u.enable_input(InpSel.SRC_1, 1)      # lane 0: g_{2k}
    u.enable_input(InpSel.SRC_0_HI, 2)   # lane 1: rho_{2k-1}
    u.enable_input(InpSel.SRC_1_HI, 3)   # lane 2: g_{2k+1}
    u.enable_input(InpSel.CONST_0, 4)    # lane 3: s0 = 0.5
    dp = u.datapath_config

    # blk0: a0 = rho_{2k-1} * C0
    dp[0].enable_alu(AluOp.MULTIPLY, A.PREV_DELAY_1, A.PREV_DELAY_3)
    dp[0].pass_through_delay(0, 1, 2, 3)
    # blk1: o0 = a0 + g_{2k}
    dp[1].enable_alu(AluOp.ADD, A.PREV_ALU_OUT, A.PREV_DELAY_0)
    dp[1].pass_through_delay(2, 3)
    # blk2: a1 = o0 * C0 ; capture o0 -> L4
    dp[2].enable_alu(AluOp.MULTIPLY, A.PREV_ALU_OUT, A.PREV_DELAY_3)
    dp[2].pass_through_delay(2, 3)
    dp[2].enable_delay_from_src(DelayInp.PREV_ALU_OUT, 4)
    # blk3: o1 = a1 + g_{2k+1}
    dp[3].enable_alu(AluOp.ADD, A.PREV_ALU_OUT, A.PREV_DELAY_2)
    dp[3].pass_through_delay(4)
    # blk4..7: carry o1 on the ALU chain, o0 on L4
    for b in (4, 5, 6, 7):
        dp[b].pass_through_alu()
        dp[b].pass_through_delay(4)

    u.enable_output(OutSel.DELAY_4, OutPath.WR0_LO)   # rho_{2k}
    u.enable_output(OutSel.ALU_OUT, OutPath.WR0_HI)   # rho_{2k+1}
    u.require_inp0 = 1
    u.require_inp1 = 1
    u.trigger = (Trigger.SRC_TENSOR_DONE, Trigger.NONE, Trigger.NONE)
    return u


def _register_ops():
    """Register the fused custom DVE ops (idempotent)."""
    import concourse.dve_ops as dve_ops
    from concourse.dve_ops import _COMPILE_CACHE
    from concourse.dve_spec import C0, Spec, Src0, Src1, lower
    from concourse.dve_uop import DveOpSpec

    def reg(name, spec, uops_2x=None):
        for o in dve_ops.OPS:
            if o.name == name:
                return o
        row = max(dve_ops._SUB_OPCODE_FOR_NAME.values()) + 1
        assert row < 0x20
        shas = {}
        for ver in ("v3", "v4"):
            kw = {}
            if uops_2x is not None and ver == "v3":
                kw = dict(uops_2x=[uops_2x], perf_max=1)
            d = DveOpSpec(name=name, opcode=row, uops=lower(spec, ver=ver),
                          rd1_en=True, **kw)
            shas[ver] = d.sha(ver)
            _COMPILE_CACHE[(name, ver)] = d
        op = dve_ops.DveOp(name, spec, subdim=False, uops_sha=shas)
        dve_ops.OPS.append(op)
        dve_ops.CUSTOM_DVE_SPECS[name] = spec
        dve_ops._SUB_OPCODE_FOR_NAME[name] = row
        return op

    lif = reg(
        "LIF_STEP_ANT",
        Spec(
            body=Src0 * (Src0 < C0) + Src1,
            reference=lambda in0, in1, s0, s1, imm2: (
                in0 * (in0 < s0) + in1
            ).astype(np.float32),
        ),
    )
    # NOTE: the 1x body below is a placeholder for the table's REGULAR
    # slot; the kernel always runs the 2x pair program (even counts,
    # bf16, aligned, perf_max=1).  The pair semantics are not
    # expressible at one element/cycle.
    resp = reg(
        "RESPAIR_ANT",
        Spec(
            body=(Src0 * C0 + Src1) * C0,
            reference=lambda in0, in1, s0, s1, imm2: (
                (in0 * s0 + in1) * s0
            ).astype(np.float32),
        ),
        uops_2x=_build_res_pair_2x_uop(),
    )
    return lif, resp


def _build_program():
    import concourse.bacc as bacc
    import concourse.mybir as mybir
    from concourse.tile import TileContext

    f32 = mybir.dt.float32
    bf16 = mybir.dt.bfloat16
    Act = mybir.ActivationFunctionType
    lif, resp = _register_ops()

    nc = bacc.Bacc()
    x_d = nc.dram_tensor("x", [P, N_STEPS * F], f32, kind="ExternalInput")
    o_d = nc.dram_tensor("o", [P, N_STEPS * F], bf16, kind="ExternalOutput")

    with TileContext(nc) as tc:
        with (
            tc.tile_pool(name="xin", bufs=1) as xpool,
            tc.tile_pool(name="single", bufs=1) as spool,
        ):
            M = spool.tile([P, N_STEPS + 1, F], f32)    # membrane ring
            # pair ring: row k = (rho_{2k}, rho_{2k+1}) interleaved; row 0
            # = (rho_{-2}, rho_{-1}) = -2
            R = spool.tile([P, NPAIR + 1, 2 * F], bf16)
            neg1 = spool.tile([P, 1], f32)              # Sign bias
            nc.vector.memset(M[:, 0, :], 0.0)
            nc.vector.memset(R[:, 0, :], -2.0)
            nc.gpsimd.memset(neg1[:], -1.0)

            xts, sts = [], []
            for ci, w in enumerate(IN_CHUNKS):
                t0 = STARTS[ci]
                xt = xpool.tile([P, w, F], f32, name=f"xin{ci}",
                                tag=f"xin{ci}")
                nc.sync.dma_start(
                    out=xt[:], in_=x_d[:, t0 * F:(t0 + w) * F]
                )
                xts.append(xt)
                # pair-interleaved sign chunk: [w/2 pair-rows, F, 2]
                st = xpool.tile([P, w // 2, 2 * F], bf16, name=f"sgn{ci}",
                                tag=f"sgn{ci}")
                sts.append(st)

            def emit_lif(ci):
                t0, w = STARTS[ci], IN_CHUNKS[ci]
                nc.vector._custom_dve(
                    lif, out=M[:, t0 + 1:t0 + 1 + w, :],
                    in0=M[:, t0:t0 + w, :], in1=xts[ci][:], s0=1.0,
                )

            def emit_sign(ci):
                t0, w = STARTS[ci], IN_CHUNKS[ci]
                st = sts[ci]
                stv = st[:].rearrange("p r (f two) -> p r f two", two=2)
                mv = M[:, t0 + 1:t0 + 1 + w, :].rearrange(
                    "p (r two) f -> p r two f", two=2)
                # even steps -> parity slot 0, odd steps -> slot 1
                for q in (0, 1):
                    nc.scalar.activation(
                        stv[:, :, :, q], mv[:, :, q, :], Act.Sign,
                        bias=neg1[:], scale=1.0,
                    )

            def emit_res(ci):
                t0, w = STARTS[ci], IN_CHUNKS[ci]
                k0 = t0 // 2
                bi = nc.vector._custom_dve(
                    resp, out=R[:, k0 + 1:k0 + 1 + w // 2, :],
                    in0=R[:, k0:k0 + w // 2, :], in1=sts[ci][:], s0=0.5,
                )
                bi.ins.perf_max = 1
                if ci in OUT_AFTER:
                    lo, hi = OUT_AFTER[ci]
                    nc.sync.dma_start(
                        out=o_d[:, lo * F:hi * F],
                        in_=R[:, lo // 2 + 1:hi // 2 + 1, :],
                    )

            # software-pipelined by one chunk
            emit_lif(0)
            for ci in range(1, len(IN_CHUNKS)):
                emit_lif(ci)
                emit_sign(ci - 1)
                emit_res(ci - 1)
            emit_sign(len(IN_CHUNKS) - 1)
            emit_res(len(IN_CHUNKS) - 1)
    nc.finalize()
    return nc


def _get_program():
    if "nc" not in _CACHE:
        _CACHE["nc"] = _build_program()
    return _CACHE["nc"]


def _shard_inputs(x: np.ndarray) -> list[np.ndarray]:
    """[32,1024,512] -> per-core [P, N_STEPS*F] partition-major arrays."""
    xf = np.ascontiguousarray(x, dtype=np.float32).reshape(N_STEPS, N_NEURONS)
    shards = []
    for c in range(N_CORES):
        s = xf[:, c * N_PER_CORE:(c + 1) * N_PER_CORE]   # [T, 16384]
        s = s.reshape(N_STEPS, P, F).transpose(1, 0, 2).reshape(
            P, N_STEPS * F
        )
        shards.append(np.ascontiguousarray(s))
    return shards


def _unshard_outputs(outs: list[np.ndarray]) -> np.ndarray:
    """Per-core rho pairs [P, NPAIR*2F] bf16 -> res [32,1024,512] f32."""
    full = np.empty((N_STEPS, N_NEURONS), dtype=np.float32)
    for c, o in enumerate(outs):
        s = np.asarray(o).astype(np.float32) * 0.5 + 1.0
        # [P, NPAIR, F, 2] -> [t, p, f]
        s = s.reshape(P, NPAIR, F, 2).transpose(1, 3, 0, 2).reshape(
            N_STEPS, N_PER_CORE
        )
        full[:, c * N_PER_CORE:(c + 1) * N_PER_CORE] = s
    return full.reshape(32, 1024, 512)


def kernel(x: np.ndarray) -> np.ndarray:
    from concourse.bass_utils import run_bass_kernel_spmd

    steps, tb, d = x.shape
    assert (steps, tb, d) == (32, 1024, 512), x.shape

    in_maps = [{"x": s} for s in _shard_inputs(x)]
    nc = _get_program()
    res = run_bass_kernel_spmd(nc, in_maps, list(range(N_CORES)))
    return _unshard_outputs(
        [res.results[c]["o"] for c in range(N_CORES)]
    )


# revision 12
# speedup vs baseline: 1.2135x; 1.0123x over previous
"""LIF-with-residue Trainium2 kernel (v8).

Reference semantics (T=4, THRESH=1, TAU=1, ALPHA=0.5):
    x: [32, 1024, 512] fp32 -> flat timeline [128 steps, 256, 512]
    per step t:
        mem   = mem + x_t
        sp    = (mem >= 1.0)
        res   = 0.5 * res + sp          # output at step t
        mem   = mem * (1 - sp)

Per core: 16384 neurons = 128 partitions x 128 f, 128 steps.

With g_t = sign(mem_t - 1) (exact threshold: (g>=0.5) == (mem>=1)) and
rho_t := 2*res_t - 2, the residue recurrence is

    rho_t = 0.5*rho_{t-1} + g_t ,  rho_{-1} = -2,  res = rho/2 + 1 (host)

v8 runs it on the DVE in 2x packed mode with a PAIR-INTERLEAVED ring:
ring row k stores (rho_{2k}[f], rho_{2k+1}[f]) adjacent, so one packed
32-bit read/write handles one neuron's two consecutive steps:

    o0 = rho_{2k}   = 0.5*SRC_0_HI(rho_{2k-1}) + SRC_1(g_{2k})
    o1 = rho_{2k+1} = 0.5*o0                   + SRC_1_HI(g_{2k+1})

The intra-pair dependency chains through the pipe (PREV_ALU_OUT); the
ring write->read distance is 2F = 256 elements, above the ~100-cycle
2x-mode read-ahead hazard window measured on HW (fails <= 192 elems,
passes >= 224).  The sign tensor is produced pair-interleaved by two
strided ScalarE Sign passes (even rows / odd rows) per chunk.

Engines: DVE = LIF (fp32 1x) + RES pair op (bf16 2x); ACT = Sign x2;
Sync = all DMAs (inputs up-front, outputs per chunk).  Host output map:
res = rho*0.5 + 1 with pair de-interleave, folded into the upcast.

Sharding: neuron n_core = p*128 + f; core c owns neurons
[c*16384, (c+1)*16384) -- data-parallel, no cross-core comms.
"""

import numpy as np

N_STEPS = 128
N_NEURONS = 131072
N_CORES = 8
N_PER_CORE = N_NEURONS // N_CORES   # 16384
P = 128                             # SBUF partitions
F = N_PER_CORE // P                 # 128 neurons per partition
NPAIR = N_STEPS // 2                # 64 pair-rows

_CACHE = {}

IN_CHUNKS = [4, 8, 16, 32, 32, 16, 12, 8]   # all even
assert sum(IN_CHUNKS) == N_STEPS and all(w % 2 == 0 for w in IN_CHUNKS)
STARTS = np.cumsum([0] + IN_CHUNKS).tolist()
# output DMA after RES of chunk index -> step range [lo, hi), even bounds
OUT_AFTER = {3: (0, 28), 4: (28, 60), 5: (60, 92), 6: (92, 108),
             7: (108, 128)}


def _build_res_pair_2x_uop():
    """2x uop: packed pair = one neuron's two consecutive steps.
    o0 = SRC_0_HI*C0 + SRC_1 ; o1 = o0*C0 + SRC_1_HI.
    Lanes: L0=SRC_1, L1=SRC_0_HI, L2=SRC_1_HI, L3=CONST_0, L4=o0."""
    from concourse.dve_uop import (
        AluInp as A,
        AluOp,
        DelayInp,
        InpSel,
        OutPath,
        OutSel,
        Trigger,
        UopConfig,
    )

    u = UopConfig()
    u.enable_input(InpSel.SRC_0, 0)      # rho_{2k-2}: delivered, unused
    u.enable_input(InpSel.SRC_1, 1)      # lane 0: g_{2k}
    u.enable_input(InpSel.SRC_0_HI, 2)   # lane 1: rho_{2k-1}
    u.enable_input(InpSel.SRC_1_HI, 3)   # lane 2: g_{2k+1}
    u.enable_input(InpSel.CONST_0, 4)    # lane 3: s0 = 0.5
    dp = u.datapath_config

    # blk0: a0 = rho_{2k-1} * C0
    dp[0].enable_alu(AluOp.MULTIPLY, A.PREV_DELAY_1, A.PREV_DELAY_3)
    dp[0].pass_through_delay(0, 1, 2, 3)
    # blk1: o0 = a0 + g_{2k}
    dp[1].enable_alu(AluOp.ADD, A.PREV_ALU_OUT, A.PREV_DELAY_0)
    dp[1].pass_through_delay(2, 3)
    # blk2: a1 = o0 * C0 ; capture o0 -> L4
    dp[2].enable_alu(AluOp.MULTIPLY, A.PREV_ALU_OUT, A.PREV_DELAY_3)
    dp[2].pass_through_delay(2, 3)
    dp[2].enable_delay_from_src(DelayInp.PREV_ALU_OUT, 4)
    # blk3: o1 = a1 + g_{2k+1}
    dp[3].enable_alu(AluOp.ADD, A.PREV_ALU_OUT, A.PREV_DELAY_2)
    dp[3].pass_through_delay(4)
    # blk4..7: carry o1 on the ALU chain, o0 on L4
    for b in (4, 5, 6, 7):
        dp[b].pass_through_alu()
        dp[b].pass_through_delay(4)

    u.enable_output(OutSel.DELAY_4, OutPath.WR0_LO)   # rho_{2k}
    u.enable_output(OutSel.ALU_OUT, OutPath.WR0_HI)   # rho_{2k+1}
    u.require_inp0 = 1
    u.require_inp1 = 1
    u.trigger = (Trigger.SRC_TENSOR_DONE, Trigger.NONE, Trigger.NONE)
    return u


def _register_ops():
    """Register the fused custom DVE ops (idempotent)."""
    import concourse.dve_ops as dve_ops
    from concourse.dve_ops import _COMPILE_CACHE
    from concourse.dve_spec import C0, Spec, Src0, Src1, lower
    from concourse.dve_uop import DveOpSpec

    def reg(name, spec, uops_2x=None):
        for o in dve_ops.OPS:
            if o.name == name:
                return o
        row = max(dve_ops._SUB_OPCODE_FOR_NAME.values()) + 1
        assert row < 0x20
        shas = {}
        for ver in ("v3", "v4"):
            kw = {}
            if uops_2x is not None and ver == "v3":
                kw = dict(uops_2x=[uops_2x], perf_max=1)
            d = DveOpSpec(name=name, opcode=row, uops=lower(spec, ver=ver),
                          rd1_en=True, **kw)
            shas[ver] = d.sha(ver)
            _COMPILE_CACHE[(name, ver)] = d
        op = dve_ops.DveOp(name, spec, subdim=False, uops_sha=shas)
        dve_ops.OPS.append(op)
        dve_ops.CUSTOM_DVE_SPECS[name] = spec
        dve_ops._SUB_OPCODE_FOR_NAME[name] = row
        return op

    lif = reg(
        "LIF_STEP_ANT",
        Spec(
            body=Src0 * (Src0 < C0) + Src1,
            reference=lambda in0, in1, s0, s1, imm2: (
                in0 * (in0 < s0) + in1
            ).astype(np.float32),
        ),
    )
    # NOTE: the 1x body below is a placeholder for the table's REGULAR
    # slot; the kernel always runs the 2x pair program (even counts,
    # bf16, aligned, perf_max=1).  The pair semantics are not
    # expressible at one element/cycle.
    resp = reg(
        "RESPAIR_ANT",
        Spec(
            body=(Src0 * C0 + Src1) * C0,
            reference=lambda in0, in1, s0, s1, imm2: (
                (in0 * s0 + in1) * s0
            ).astype(np.float32),
        ),
        uops_2x=_build_res_pair_2x_uop(),
    )
    return lif, resp


def _build_program():
    import concourse.bacc as bacc
    import concourse.mybir as mybir
    from concourse.tile import TileContext

    f32 = mybir.dt.float32
    bf16 = mybir.dt.bfloat16
    Act = mybir.ActivationFunctionType
    lif, resp = _register_ops()

    nc = bacc.Bacc()
    x_d = nc.dram_tensor("x", [P, N_STEPS * F], f32, kind="ExternalInput")
    o_d = nc.dram_tensor("o", [P, N_STEPS * F], bf16, kind="ExternalOutput")

    with TileContext(nc) as tc:
        with (
            tc.tile_pool(name="xin", bufs=1) as xpool,
            tc.tile_pool(name="single", bufs=1) as spool,
        ):
            M = spool.tile([P, N_STEPS + 1, F], f32)    # membrane ring
            # pair ring: row k = (rho_{2k}, rho_{2k+1}) interleaved; row 0
            # = (rho_{-2}, rho_{-1}) = -2
            R = spool.tile([P, NPAIR + 1, 2 * F], bf16)
            neg1 = spool.tile([P, 1], f32)              # Sign bias
            nc.vector.memset(M[:, 0, :], 0.0)
            nc.vector.memset(R[:, 0, :], -2.0)
            nc.gpsimd.memset(neg1[:], -1.0)

            xts, sts = [], []
            for ci, w in enumerate(IN_CHUNKS):
                t0 = STARTS[ci]
                xt = xpool.tile([P, w, F], f32, name=f"xin{ci}",
                                tag=f"xin{ci}")
                nc.sync.dma_start(
                    out=xt[:], in_=x_d[:, t0 * F:(t0 + w) * F]
                )
                xts.append(xt)
                # pair-interleaved sign chunk: [w/2 pair-rows, F, 2]
                st = xpool.tile([P, w // 2, 2 * F], bf16, name=f"sgn{ci}",
                                tag=f"sgn{ci}")
                sts.append(st)

            def emit_lif(ci):
                t0, w = STARTS[ci], IN_CHUNKS[ci]
                nc.vector._custom_dve(
                    lif, out=M[:, t0 + 1:t0 + 1 + w, :],
                    in0=M[:, t0:t0 + w, :], in1=xts[ci][:], s0=1.0,
                )

            def emit_sign(ci):
                t0, w = STARTS[ci], IN_CHUNKS[ci]
                st = sts[ci]
                stv = st[:].rearrange("p r (f two) -> p r f two", two=2)
                mv = M[:, t0 + 1:t0 + 1 + w, :].rearrange(
                    "p (r two) f -> p r two f", two=2)
                # even steps -> parity slot 0, odd steps -> slot 1
                for q in (0, 1):
                    nc.scalar.activation(
                        stv[:, :, :, q], mv[:, :, q, :], Act.Sign,
                        bias=neg1[:], scale=1.0,
                    )

            def emit_res(ci):
                t0, w = STARTS[ci], IN_CHUNKS[ci]
                k0 = t0 // 2
                bi = nc.vector._custom_dve(
                    resp, out=R[:, k0 + 1:k0 + 1 + w // 2, :],
                    in0=R[:, k0:k0 + w // 2, :], in1=sts[ci][:], s0=0.5,
                )
                bi.ins.perf_max = 1
                if ci in OUT_AFTER:
                    lo, hi = OUT_AFTER[ci]
                    nc.sync.dma_start(
                        out=o_d[:, lo * F:hi * F],
                        in_=R[:, lo // 2 + 1:hi // 2 + 1, :],
                    )

            # software-pipelined: RES trails LIF by two chunks so the
            # two strided Sign passes never stall the DVE
            n = len(IN_CHUNKS)
            emit_lif(0)
            emit_lif(1)
            emit_sign(0)
            for ci in range(2, n):
                emit_lif(ci)
                emit_sign(ci - 1)
                emit_res(ci - 2)
            emit_sign(n - 1)
            emit_res(n - 2)
            emit_res(n - 1)
    nc.finalize()
    return nc


def _get_program():
    if "nc" not in _CACHE:
        _CACHE["nc"] = _build_program()
    return _CACHE["nc"]


def _shard_inputs(x: np.ndarray) -> list[np.ndarray]:
    """[32,1024,512] -> per-core [P, N_STEPS*F] partition-major arrays."""
    xf = np.ascontiguousarray(x, dtype=np.float32).reshape(N_STEPS, N_NEURONS)
    shards = []
    for c in range(N_CORES):
        s = xf[:, c * N_PER_CORE:(c + 1) * N_PER_CORE]   # [T, 16384]
        s = s.reshape(N_STEPS, P, F).transpose(1, 0, 2).reshape(
            P, N_STEPS * F
        )
        shards.append(np.ascontiguousarray(s))
    return shards


def _unshard_outputs(outs: list[np.ndarray]) -> np.ndarray:
    """Per-core rho pairs [P, NPAIR*2F] bf16 -> res [32,1024,512] f32."""
    full = np.empty((N_STEPS, N_NEURONS), dtype=np.float32)
    for c, o in enumerate(outs):
        s = np.asarray(o).astype(np.float32) * 0.5 + 1.0
        # [P, NPAIR, F, 2] -> [t, p, f]
        s = s.reshape(P, NPAIR, F, 2).transpose(1, 3, 0, 2).reshape(
            N_STEPS, N_PER_CORE
        )
        full[:, c * N_PER_CORE:(c + 1) * N_PER_CORE] = s
    return full.reshape(32, 1024, 512)


def kernel(x: np.ndarray) -> np.ndarray:
    from concourse.bass_utils import run_bass_kernel_spmd

    steps, tb, d = x.shape
    assert (steps, tb, d) == (32, 1024, 512), x.shape

    in_maps = [{"x": s} for s in _shard_inputs(x)]
    nc = _get_program()
    res = run_bass_kernel_spmd(nc, in_maps, list(range(N_CORES)))
    return _unshard_outputs(
        [res.results[c]["o"] for c in range(N_CORES)]
    )
